# revision 12
# baseline (speedup 1.0000x reference)
"""Bahdanau additive attention on 8 Trainium2 NeuronCores.

Shapes (hardcoded per the problem spec):
  query  [B=64, Q=1024], memory [B=64, T=2048, M=1024]
  W_query [A=512, Q], W_memory [A=512, M], W_v [1, A]
Outputs: (alpha [B, T], context [B, 1, M])

Strategy: data-parallel over batch — 8 batches per core. Each core:
  pass 1: wmT[a, t] = (W_memory @ memory[b].T) in PSUM (fp32r matmuls,
          contraction m on partitions), tanh(+wq bias) on ACT,
          v-weighted reduce over a via [1, N] matmuls -> scores.
  softmax per batch on a [1, T] row (max / exp+sum fused / normalize).
  pass 2: context[m] = sum_t alpha[t] * memT[m, t] via DVE
          tensor_tensor_reduce against a partition-broadcast alpha.

Host side: shards batch, pre-transposes memory to [M, T] per batch (layout
prep only; the device kernel does all the math), gathers per-core outputs.
"""

import os
from contextlib import ExitStack

import numpy as np

N_CORES = 8
B, T, M, Q, A = 64, 2048, 1024, 1024, 512
BL = B // N_CORES  # batches per core
MT = M // 128      # m tiles (8)
QT = Q // 128      # q tiles (8)
AT = A // 128      # a tiles (4)
TC = T // 512      # t chunks (4)

# fp32r: PE matmul mode that reads fp32 operands at ~bf16 streaming rate
# (1 cycle/row for moving dim >= 256) at reduced multiply precision.
USE_F32R = os.environ.get("BAHDANAU_F32R", "1") == "1"

_CACHE = {}


def _build_program():
    import concourse.bass as bass
    import concourse.tile as tile
    from concourse import bacc, mybir

    f32 = mybir.dt.float32
    f32r = mybir.dt.float32r
    AF = mybir.ActivationFunctionType
    ALU = mybir.AluOpType
    AX = mybir.AxisListType

    mdt = f32r if USE_F32R else f32  # matmul-path dtype (end-to-end)

    nc = bacc.Bacc(
        "TRN2",
        target_bir_lowering=False,
        debug=False,
        num_devices=N_CORES,
    )

    memT_d = nc.dram_tensor("memT", [BL, M, T], mdt, kind="ExternalInput").ap()
    qT_d = nc.dram_tensor("queryT", [Q, BL], mdt, kind="ExternalInput").ap()
    WmT_d = nc.dram_tensor("WmT", [M, A], mdt, kind="ExternalInput").ap()
    WqT_d = nc.dram_tensor("WqT", [Q, A], mdt, kind="ExternalInput").ap()
    v4_d = nc.dram_tensor("v4", [128, AT], mdt, kind="ExternalInput").ap()
    ones_d = nc.dram_tensor("ones", [1, 128], mdt, kind="ExternalInput").ap()
    alpha_d = nc.dram_tensor("alpha_out", [BL, T], f32, kind="ExternalOutput").ap()
    ctx_d = nc.dram_tensor("ctx_out", [BL, M], f32, kind="ExternalOutput").ap()

    with tile.TileContext(nc) as tc, ExitStack() as ctx:
        const_p = ctx.enter_context(tc.tile_pool(name="const", bufs=1))
        boot_p = ctx.enter_context(tc.tile_pool(name="boot", bufs=2))
        mem_p = ctx.enter_context(tc.tile_pool(name="mem", bufs=11))
        tanh_p = ctx.enter_context(tc.tile_pool(name="tanh", bufs=4))
        row_p = ctx.enter_context(tc.tile_pool(name="row", bufs=1))
        bc_p = ctx.enter_context(tc.tile_pool(name="bc", bufs=1))
        scr_p = ctx.enter_context(tc.tile_pool(name="scr", bufs=2))
        scr2_p = ctx.enter_context(tc.tile_pool(name="scr2", bufs=1))
        psum_p = ctx.enter_context(
            tc.tile_pool(name="psum", bufs=3, space="PSUM")
        )
        psc_p = ctx.enter_context(tc.tile_pool(name="psc", bufs=2, space="PSUM"))
        pwq_p = ctx.enter_context(tc.tile_pool(name="pwq", bufs=1, space="PSUM"))
        pbc_p = ctx.enter_context(tc.tile_pool(name="pbc", bufs=2, space="PSUM"))

        # ---- weights / small inputs ----
        wm_w = const_p.tile([128, MT, A], mdt)  # [p, mt, a] = WmT[mt*128+p, a]
        nc.sync.dma_start(wm_w[:], WmT_d.rearrange("(mt p) a -> p mt a", p=128))
        v_sb = const_p.tile([128, AT], mdt)
        nc.sync.dma_start(v_sb[:], v4_d[:])
        qT_sb = const_p.tile([128, QT, BL], mdt)
        nc.sync.dma_start(qT_sb[:], qT_d.rearrange("(qt p) b -> p qt b", p=128))

        # ---- wqT[a, b] = sum_q W_query[a, q] query[b, q], laid out
        #      [128a_p, AT*BL] so column at*BL+b is the tanh bias vector ----
        wq_all = boot_p.tile([128, QT, A], mdt)
        nc.sync.dma_start(wq_all[:], WqT_d.rearrange("(qt p) a -> p qt a", p=128))
        wq_sb = const_p.tile([128, AT * BL], f32)
        for at in range(AT):
            wq_ps = pwq_p.tile([128, BL], f32, tag="wqps")
            for qt in range(QT):
                nc.tensor.matmul(
                    wq_ps[:],
                    (wq_all[:, qt, at * 128:(at + 1) * 128]),
                    (qT_sb[:, qt, :]),
                    start=(qt == 0),
                    stop=(qt == QT - 1),
                )
            nc.scalar.copy(wq_sb[:, at * BL:(at + 1) * BL], wq_ps[:])

        ctx_all = const_p.tile([128, BL * MT], f32)
        ones_sb = const_p.tile([1, 128], mdt)
        nc.sync.dma_start(ones_sb[:], ones_d[:])

        # ---- main pipeline over local batches ----
        prev_sc = None  # deferred score matmul (software pipelining)
        for b in range(BL):
            mem_tiles = []
            for mt in range(MT):
                mtile = mem_p.tile([128, T], mdt, tag="mem")
                nc.sync.dma_start(
                    mtile[:], memT_d[b, mt * 128:(mt + 1) * 128, :]
                )
                mem_tiles.append(mtile)

            scores = row_p.tile([1, T], f32, tag="scores")
            psc = None
            for g in range(TC * AT):
                tc_i, at = divmod(g, AT)
                if at == 0:
                    psc = psc_p.tile([1, 512], f32, tag="psc")
                wm_ps = psum_p.tile([128, 512], f32, tag="wm")
                for mt in range(MT):
                    nc.tensor.matmul(
                        wm_ps[:],
                        (wm_w[:, mt, at * 128:(at + 1) * 128]),
                        (mem_tiles[mt][:, tc_i * 512:(tc_i + 1) * 512]),
                        start=(mt == 0),
                        stop=(mt == MT - 1),
                    )
                th = tanh_p.tile([128, 512], mdt, tag="tanh")
                nc.scalar.activation(
                    th[:], wm_ps[:], AF.Tanh,
                    bias=wq_sb[:, at * BL + b:at * BL + b + 1],
                )

                def emit_sc(th=th, psc=psc, at=at, tc_i=tc_i, scores=scores):
                    nc.tensor.matmul(
                        psc[:],
                        (v_sb[:, at:at + 1]),
                        (th[:]),
                        start=(at == 0),
                        stop=(at == AT - 1),
                    )
                    if at == AT - 1:
                        # psum group complete -> move the score row to SBUF
                        nc.scalar.copy(
                            scores[:, tc_i * 512:(tc_i + 1) * 512], psc[:]
                        )

                if prev_sc is not None:
                    prev_sc()
                prev_sc = emit_sc
            # flush last score matmul of this batch
            prev_sc()
            prev_sc = None

            # softmax on the [1, T] row
            mx = row_p.tile([1, 1], f32, tag="mx")
            nc.vector.reduce_max(mx[:], scores[:], axis=AX.X)
            nmx = row_p.tile([1, 1], f32, tag="nmx")
            nc.vector.tensor_scalar_mul(nmx[:], mx[:], -1.0)
            alpha_u = row_p.tile([1, T], f32, tag="alpha_u")
            sume = row_p.tile([1, 1], f32, tag="sume")
            nc.scalar.activation(
                alpha_u[:], scores[:], AF.Exp, bias=nmx[:], accum_out=sume[:]
            )
            rcp = row_p.tile([1, 1], f32, tag="rcp")
            nc.vector.reciprocal(rcp[:], sume[:])
            alpha_n = row_p.tile([1, T], mdt, tag="alpha_n")
            nc.vector.tensor_scalar_mul(alpha_n[:], alpha_u[:], rcp[:])
            nc.sync.dma_start(alpha_d[b:b + 1, :], alpha_n[:].bitcast(f32))

            # pass 2: context[m] = sum_t alpha[t] * memT[m, t].
            # Broadcast alpha across partitions with a ones[1,128] outer
            # product on the PE, then reduce on the DVE.
            bc = bc_p.tile([128, T], f32, tag="bc")
            for tcn in range(TC):
                pbc = pbc_p.tile([128, 512], f32, tag="pbc")
                nc.tensor.matmul(
                    pbc[:], ones_sb[:],
                    alpha_n[:, tcn * 512:(tcn + 1) * 512],
                )
                nc.vector.tensor_copy(
                    bc[:, tcn * 512:(tcn + 1) * 512], pbc[:]
                )
            for mt in range(MT):
                # DVE: products; ACT: free-dim reduction via accum_out
                # (InstTensorTensorReduce is broken on this HW path).
                scr = scr_p.tile([128, T], f32, tag="scr")
                nc.vector.tensor_tensor(
                    scr[:], mem_tiles[mt][:].bitcast(f32), bc[:], ALU.mult
                )
                scr2 = scr2_p.tile([128, T], f32, tag="scr2")
                nc.scalar.activation(
                    scr2[:], scr[:], AF.Copy,
                    accum_out=ctx_all[:, b * MT + mt:b * MT + mt + 1],
                )

        nc.sync.dma_start(
            ctx_d.rearrange("b (mt p) -> p b mt", p=128),
            ctx_all[:].rearrange("p (b mt) -> p b mt", b=BL),
        )

    nc.compile()
    return nc


def _get_program():
    if "nc" not in _CACHE:
        _CACHE["nc"] = _build_program()
    return _CACHE["nc"]


def kernel(query, memory, W_query, W_memory, W_v):
    from concourse.bass_utils import run_bass_kernel_spmd

    nc = _get_program()

    WmT = np.ascontiguousarray(W_memory.T)          # [M, A]
    WqT = np.ascontiguousarray(W_query.T)           # [Q, A]
    v4 = np.ascontiguousarray(W_v[0].reshape(AT, 128).T)  # [128, AT]

    in_maps = []
    for c in range(N_CORES):
        sl = slice(c * BL, (c + 1) * BL)
        in_maps.append({
            "memT": np.ascontiguousarray(memory[sl].transpose(0, 2, 1)),
            "queryT": np.ascontiguousarray(query[sl].T),
            "WmT": WmT,
            "WqT": WqT,
            "v4": v4,
            "ones": np.ones((1, 128), np.float32),
        })

    trace = os.environ.get("BAHDANAU_TRACE", "0") == "1"
    res = run_bass_kernel_spmd(nc, in_maps, list(range(N_CORES)), trace=trace)
    _CACHE["last_results"] = res
    alpha = np.concatenate([r["alpha_out"] for r in res.results], axis=0)
    context = np.concatenate([r["ctx_out"] for r in res.results], axis=0)
    return alpha.astype(np.float32), context[:, None, :].astype(np.float32)


# revision 13
# speedup vs baseline: 1.4909x; 1.4909x over previous
"""Bahdanau additive attention on 8 Trainium2 NeuronCores.

Shapes (hardcoded per the problem spec):
  query  [B=64, Q=1024], memory [B=64, T=2048, M=1024]
  W_query [A=512, Q], W_memory [A=512, M], W_v [1, A]
Outputs: (alpha [B, T], context [B, 1, M])

Strategy: data-parallel over batch — 8 batches per core. Each core:
  pass 1: wmT[a, t] = (W_memory @ memory[b].T) in PSUM (contraction m on
          partitions), tanh(+wq bias) on ACT, v-weighted reduce over a via
          [1, N] matmuls -> scores.
  softmax per batch on a [1, T] row (max / exp+sum fused / normalize).
  pass 2: context[m] = sum_t alpha[t] * memT[m, t]: alpha broadcast across
          partitions via a ones[1,128] PE outer product, DVE multiply,
          ACT copy-with-accumulate for the free-dim reduction.

Matmul dtype (BAHDANAU_DTYPE):
  f16  — fp16 operands: 1 cyc/row stream + background-loadable LDWEIGHTS
         (hidden under MATMUL), half HBM. ~5e-4 rel err.
  f32r — fp32 operands in reduced-precision mode: 1 cyc/row but every
         matmul self-loads weights (~+190ns serial each). ~2.5e-4 rel err.

Host side: shards batch, pre-transposes memory to [M, T] per batch (layout
prep only; the device kernel does all the math), gathers per-core outputs.
"""

import os
from contextlib import ExitStack

import numpy as np

N_CORES = 8
B, T, M, Q, A = 64, 2048, 1024, 1024, 512
BL = B // N_CORES  # batches per core
MT = M // 128      # m tiles (8)
QT = Q // 128      # q tiles (8)
AT = A // 128      # a tiles (4)
TC = T // 512      # t chunks (4)

DTYPE_MODE = os.environ.get("BAHDANAU_DTYPE", "f16")

_CACHE = {}


def _build_program():
    import concourse.bass as bass  # noqa: F401
    import concourse.tile as tile
    from concourse import bacc, mybir

    f32 = mybir.dt.float32
    AF = mybir.ActivationFunctionType
    ALU = mybir.AluOpType
    AX = mybir.AxisListType

    f16_mode = DTYPE_MODE == "f16"
    mdt = mybir.dt.float16 if f16_mode else mybir.dt.float32r
    # dtype for the context (pass 2) elementwise path
    cdt = mybir.dt.float16 if f16_mode else f32

    def dve_view(ap):
        # non-matmul engines read f32r tiles as plain f32
        return ap if f16_mode else ap.bitcast(f32)

    nc = bacc.Bacc(
        "TRN2",
        target_bir_lowering=False,
        debug=False,
        num_devices=N_CORES,
    )

    memT_d = nc.dram_tensor("memT", [BL, M, T], mdt, kind="ExternalInput").ap()
    qT_d = nc.dram_tensor("queryT", [Q, BL], mdt, kind="ExternalInput").ap()
    WmT_d = nc.dram_tensor("WmT", [M, A], mdt, kind="ExternalInput").ap()
    WqT_d = nc.dram_tensor("WqT", [Q, A], mdt, kind="ExternalInput").ap()
    v4_d = nc.dram_tensor("v4", [128, AT], mdt, kind="ExternalInput").ap()
    ones_d = nc.dram_tensor("ones", [1, 128], mdt, kind="ExternalInput").ap()
    alpha_d = nc.dram_tensor("alpha_out", [BL, T], f32, kind="ExternalOutput").ap()
    ctx_d = nc.dram_tensor("ctx_out", [BL, M], f32, kind="ExternalOutput").ap()

    with tile.TileContext(nc) as tc, ExitStack() as ctx:
        const_p = ctx.enter_context(tc.tile_pool(name="const", bufs=1))
        boot_p = ctx.enter_context(tc.tile_pool(name="boot", bufs=2))
        mem_p = ctx.enter_context(tc.tile_pool(name="mem", bufs=18))
        tanh_p = ctx.enter_context(tc.tile_pool(name="tanh", bufs=6))
        row_p = ctx.enter_context(tc.tile_pool(name="row", bufs=2))
        bc_p = ctx.enter_context(tc.tile_pool(name="bc", bufs=2))
        scr_p = ctx.enter_context(tc.tile_pool(name="scr", bufs=2))
        scr2_p = ctx.enter_context(tc.tile_pool(name="scr2", bufs=1))
        psum_p = ctx.enter_context(
            tc.tile_pool(name="psum", bufs=3, space="PSUM")
        )
        psc_p = ctx.enter_context(tc.tile_pool(name="psc", bufs=2, space="PSUM"))
        pwq_p = ctx.enter_context(tc.tile_pool(name="pwq", bufs=1, space="PSUM"))
        pbc_p = ctx.enter_context(tc.tile_pool(name="pbc", bufs=2, space="PSUM"))

        # ---- weights / small inputs ----
        wm_w = const_p.tile([128, MT, A], mdt)  # [p, mt, a] = WmT[mt*128+p, a]
        nc.sync.dma_start(wm_w[:], WmT_d.rearrange("(mt p) a -> p mt a", p=128))
        v_sb = const_p.tile([128, AT], mdt)
        nc.sync.dma_start(v_sb[:], v4_d[:])
        qT_sb = const_p.tile([128, QT, BL], mdt)
        nc.sync.dma_start(qT_sb[:], qT_d.rearrange("(qt p) b -> p qt b", p=128))
        ones_sb = const_p.tile([1, 128], mdt)
        nc.sync.dma_start(ones_sb[:], ones_d[:])

        # ---- wqT[a, b] = sum_q W_query[a, q] query[b, q], laid out
        #      [128a_p, AT*BL] so column at*BL+b is the tanh bias vector ----
        wq_all = boot_p.tile([128, QT, A], mdt)
        nc.sync.dma_start(wq_all[:], WqT_d.rearrange("(qt p) a -> p qt a", p=128))
        wq_sb = const_p.tile([128, AT * BL], f32)
        for at in range(AT):
            wq_ps = pwq_p.tile([128, BL], f32, tag="wqps")
            for qt in range(QT):
                nc.tensor.matmul(
                    wq_ps[:],
                    wq_all[:, qt, at * 128:(at + 1) * 128],
                    qT_sb[:, qt, :],
                    start=(qt == 0),
                    stop=(qt == QT - 1),
                )
            nc.scalar.copy(wq_sb[:, at * BL:(at + 1) * BL], wq_ps[:])

        ctx_all = const_p.tile([128, BL * MT], f32)

        # ---- main pipeline over local batches ----
        prev_sc = None  # deferred score matmul (software pipelining)
        for b in range(BL):
            mem_tiles = []
            for mt in range(MT):
                mtile = mem_p.tile([128, T], mdt, tag="mem")
                nc.sync.dma_start(
                    mtile[:], memT_d[b, mt * 128:(mt + 1) * 128, :]
                )
                mem_tiles.append(mtile)

            scores = row_p.tile([1, T], f32, tag="scores")
            psc = None
            for g in range(TC * AT):
                tc_i, at = divmod(g, AT)
                if at == 0:
                    psc = psc_p.tile([1, 512], f32, tag="psc")
                wm_ps = psum_p.tile([128, 512], f32, tag="wm")
                for mt in range(MT):
                    nc.tensor.matmul(
                        wm_ps[:],
                        wm_w[:, mt, at * 128:(at + 1) * 128],
                        mem_tiles[mt][:, tc_i * 512:(tc_i + 1) * 512],
                        start=(mt == 0),
                        stop=(mt == MT - 1),
                    )
                th = tanh_p.tile([128, 512], mdt, tag="tanh")
                nc.scalar.activation(
                    th[:], wm_ps[:], AF.Tanh,
                    bias=wq_sb[:, at * BL + b:at * BL + b + 1],
                )

                def emit_sc(th=th, psc=psc, at=at, tc_i=tc_i, scores=scores):
                    nc.tensor.matmul(
                        psc[:],
                        v_sb[:, at:at + 1],
                        th[:],
                        start=(at == 0),
                        stop=(at == AT - 1),
                    )
                    if at == AT - 1:
                        # psum group complete -> move the score row to SBUF
                        nc.scalar.copy(
                            scores[:, tc_i * 512:(tc_i + 1) * 512], psc[:]
                        )

                if prev_sc is not None:
                    prev_sc()
                prev_sc = emit_sc
            # flush last score matmul of this batch
            prev_sc()
            prev_sc = None

            # softmax on the [1, T] row
            mx = row_p.tile([1, 1], f32, tag="mx")
            nc.vector.reduce_max(mx[:], scores[:], axis=AX.X)
            nmx = row_p.tile([1, 1], f32, tag="nmx")
            nc.vector.tensor_scalar_mul(nmx[:], mx[:], -1.0)
            alpha_u = row_p.tile([1, T], f32, tag="alpha_u")
            sume = row_p.tile([1, 1], f32, tag="sume")
            nc.scalar.activation(
                alpha_u[:], scores[:], AF.Exp, bias=nmx[:], accum_out=sume[:]
            )
            rcp = row_p.tile([1, 1], f32, tag="rcp")
            nc.vector.reciprocal(rcp[:], sume[:])
            alpha_n = row_p.tile([1, T], f32, tag="alpha_n")
            nc.vector.tensor_scalar_mul(alpha_n[:], alpha_u[:], rcp[:])
            nc.sync.dma_start(alpha_d[b:b + 1, :], alpha_n[:])
            # matmul-dtype copy of alpha for the broadcast matmul
            alpha_h = row_p.tile([1, T], mdt, tag="alpha_h")
            nc.vector.tensor_copy(alpha_h[:], alpha_n[:])

            # pass 2: context[m] = sum_t alpha[t] * memT[m, t].
            # Broadcast alpha across partitions with a ones[1,128] PE outer
            # product; multiply on DVE; reduce on ACT accum_out
            # (InstTensorTensorReduce is broken on this HW path).
            bc = bc_p.tile([128, T], cdt, tag="bc")
            for tcn in range(TC):
                pbc = pbc_p.tile([128, 512], f32, tag="pbc")
                nc.tensor.matmul(
                    pbc[:], ones_sb[:],
                    alpha_h[:, tcn * 512:(tcn + 1) * 512],
                )
                nc.vector.tensor_copy(
                    bc[:, tcn * 512:(tcn + 1) * 512], pbc[:]
                )
            for mt in range(MT):
                scr = scr_p.tile([128, T], cdt, tag="scr")
                nc.vector.tensor_tensor(
                    scr[:], dve_view(mem_tiles[mt][:]), bc[:], ALU.mult
                )
                scr2 = scr2_p.tile([128, T], cdt, tag="scr2")
                nc.scalar.activation(
                    scr2[:], scr[:], AF.Copy,
                    accum_out=ctx_all[:, b * MT + mt:b * MT + mt + 1],
                )

        nc.sync.dma_start(
            ctx_d.rearrange("b (mt p) -> p b mt", p=128),
            ctx_all[:].rearrange("p (b mt) -> p b mt", b=BL),
        )

    nc.compile()
    return nc


def _get_program():
    if "nc" not in _CACHE:
        _CACHE["nc"] = _build_program()
    return _CACHE["nc"]


def _np_dt():
    return np.float16 if DTYPE_MODE == "f16" else np.float32


def kernel(query, memory, W_query, W_memory, W_v):
    from concourse.bass_utils import run_bass_kernel_spmd

    nc = _get_program()
    ndt = _np_dt()

    WmT = np.ascontiguousarray(W_memory.T).astype(ndt)          # [M, A]
    WqT = np.ascontiguousarray(W_query.T).astype(ndt)           # [Q, A]
    v4 = np.ascontiguousarray(W_v[0].reshape(AT, 128).T).astype(ndt)
    ones = np.ones((1, 128), ndt)

    in_maps = []
    for c in range(N_CORES):
        sl = slice(c * BL, (c + 1) * BL)
        in_maps.append({
            "memT": memory[sl].transpose(0, 2, 1).astype(ndt),
            "queryT": np.ascontiguousarray(query[sl].T).astype(ndt),
            "WmT": WmT,
            "WqT": WqT,
            "v4": v4,
            "ones": ones,
        })

    trace = os.environ.get("BAHDANAU_TRACE", "0") == "1"
    res = run_bass_kernel_spmd(nc, in_maps, list(range(N_CORES)), trace=trace)
    _CACHE["last_results"] = res
    alpha = np.concatenate([r["alpha_out"] for r in res.results], axis=0)
    context = np.concatenate([r["ctx_out"] for r in res.results], axis=0)
    return alpha.astype(np.float32), context[:, None, :].astype(np.float32)
